# revision 9
# baseline (speedup 1.0000x reference)
"""Trainium2 Bass kernel for InterpolativeUpsampler.

Op: nearest 2x upsample (H, W) followed by depthwise 3x3 blur
([1,2,1] outer [1,2,1] / 16, padding=1) on NCHW fp32.

The composite op is separable per axis:
    out[2i]   = (x[i-1] + 3*x[i]) / 4      (x[-1] = 0)
    out[2i+1] = (3*x[i] + x[i+1]) / 4      (x[H]  = 0)

Strategy: pure data parallel over batch (16 samples -> 8 cores, 2 each).
Per core: channels (128) on SBUF partitions; H tiled with 1-row halo.

Memory regime: input is prescaled by 1/16, zero-padded by 1 on H/W and
cast to fp16 on the host; the device writes fp16 output that the host
casts back to fp32. This halves HBM traffic vs fp32 (rel err ~1e-3,
gate is 2e-2) and makes every tile uniform (no edge cases on device).

Compute split (scalar_tensor_tensor has no fast DVE modes; tensor_tensor
gets 2x and tensor_scalar 4x for packed unit-stride fp16):
  ACT:    q3 = 3*x (feeds gpsimd) and t3 = 3*y (feeds the H adds)
  GPSIMD: W-pass even columns   y[2j]   = q3[j] + x[j-1]   (tensor_add)
  DVE:    W-pass odd columns    y[2j+1] = 3*x[j] + x[j+1]  (STT, 1x)
          H-pass                out[2i]   = t3[i] + y[i-1]  (TT, 2x)
                                out[2i+1] = t3[i] + y[i+1]  (TT, 2x)
  SP:     input DMA; ACT: output DMA.
"""

import numpy as np

B, C, H, W = 16, 128, 128, 128
N_CORES = 8
B_LOC = B // N_CORES      # samples per core
HB = 16                   # input rows per h-tile
NT = H // HB              # h-tiles per sample
R = HB + 2                # rows incl halo
WP = W + 2                # padded width

_cache = {}


def _build(opts: dict | None = None):
    import concourse.bacc as bacc
    import concourse.mybir as mybir
    import concourse.tile as tile

    o = {
        "q3_eng": "scalar",       # q3 = 3*x (feeds gpsimd W-even)
        "w_even_eng": "gpsimd",   # W-pass even-col add (strided write)
        "w_odd_eng": "vector",    # W-pass odd-col STT (strided write, 1x)
        "t3_eng": "scalar",       # t3 = 3*y
        "h_even_eng": "vector",   # H-pass adds (fp16 2x mode)
        "h_odd_eng": "vector",
        "in_dma_eng": "sync",
        "out_dma_eng": "scalar",
        "bufs": 3,
    }
    o.update(opts or {})

    f16 = mybir.dt.float16
    mult = mybir.AluOpType.mult
    add = mybir.AluOpType.add

    nc = bacc.Bacc("TRN2", target_bir_lowering=False, debug=False,
                   num_devices=N_CORES)
    eng = {"vector": nc.vector, "gpsimd": nc.gpsimd, "sync": nc.sync,
           "scalar": nc.scalar, "tensor": nc.tensor}
    x = nc.dram_tensor("x", [B_LOC, C, H + 2, WP], f16,
                       kind="ExternalInput").ap()
    y = nc.dram_tensor("y", [B_LOC, C, 2 * H, 2 * W], f16,
                       kind="ExternalOutput").ap()

    def emul(e, out, in_, s):
        """out = in_ * s on engine e (ACT activation or DVE/gpsimd TS)."""
        if e is nc.scalar:
            e.mul(out, in_, s)
        else:
            e.tensor_scalar_mul(out, in_, s)

    with tile.TileContext(nc) as tc:
        with tc.tile_pool(name="px", bufs=o["bufs"]) as px, \
             tc.tile_pool(name="pq", bufs=o["bufs"]) as pq, \
             tc.tile_pool(name="py", bufs=o["bufs"]) as py, \
             tc.tile_pool(name="pt", bufs=o["bufs"]) as pt, \
             tc.tile_pool(name="po", bufs=o["bufs"]) as po:
            for b in range(B_LOC):
                for t in range(NT):
                    h0 = t * HB
                    xt = px.tile([C, R * WP], f16)
                    q3t = pq.tile([C, R * WP], f16)
                    yt = py.tile([C, R * 2 * W], f16)
                    t3 = pt.tile([C, HB * 2 * W], f16)
                    ot = po.tile([C, HB * 4 * W], f16)

                    xv = xt.rearrange("c (r w) -> c r w", w=WP)
                    qv = q3t.rearrange("c (r w) -> c r w", w=WP)
                    yv = yt.rearrange("c (r w) -> c r w", w=2 * W)
                    # stride-2 views of yt for interleaved W-pass writes
                    yv2 = yt.rearrange("c (r w two) -> c r w two", w=W, two=2)
                    tv = t3.rearrange("c (r w) -> c r w", w=2 * W)
                    # output rows interleaved by parity
                    ov = ot.rearrange("c (r two w) -> c r two w",
                                      two=2, w=2 * W)

                    # ---- load R=18 padded input rows (always uniform) ----
                    eng[o["in_dma_eng"]].dma_start(xv, x[b][:, h0:h0 + R, :])

                    # ---- W pass (x is pre-scaled by 1/16, zero-padded) ----
                    # q3 = 3*x
                    emul(eng[o["q3_eng"]], qv, xv, 3.0)
                    # even: y[r, 2j] = 3*x[r, j] + x[r, j-1]
                    eng[o["w_even_eng"]].tensor_add(
                        yv2[:, :, :, 0], qv[:, :, 1:W + 1], xv[:, :, 0:W])
                    # odd: y[r, 2j+1] = 3*x[r, j] + x[r, j+1]
                    eng[o["w_odd_eng"]].scalar_tensor_tensor(
                        yv2[:, :, :, 1], xv[:, :, 1:W + 1], 3.0,
                        xv[:, :, 2:W + 2], op0=mult, op1=add)

                    # ---- t3 = 3 * y (center rows) ----
                    emul(eng[o["t3_eng"]], tv, yv[:, 1:HB + 1, :], 3.0)

                    # ---- H pass: fp16 tensor_add (2x packed mode) ----
                    # out[2i] = t3[i] + y[i-1]   (tile rows i -> yv row i+1)
                    eng[o["h_even_eng"]].tensor_add(
                        ov[:, :, 0, :], tv[:, :, :], yv[:, 0:HB, :])
                    # out[2i+1] = t3[i] + y[i+1]
                    eng[o["h_odd_eng"]].tensor_add(
                        ov[:, :, 1, :], tv[:, :, :], yv[:, 2:HB + 2, :])

                    # ---- store 2*HB output rows (contiguous in HBM) ----
                    eng[o["out_dma_eng"]].dma_start(
                        y[b][:, 2 * h0:2 * h0 + 2 * HB, :],
                        ot.rearrange("c (h w) -> c h w", w=2 * W))

    nc.compile()
    return nc


def _get_nc():
    if "nc" not in _cache:
        _cache["nc"] = _build()
    return _cache["nc"]


def _in_maps(x: np.ndarray) -> list:
    xs = (np.asarray(x, dtype=np.float32) * (1.0 / 16.0)).astype(np.float16)
    xp = np.zeros((B, C, H + 2, WP), dtype=np.float16)
    xp[:, :, 1:H + 1, 1:W + 1] = xs
    return [{"x": np.ascontiguousarray(xp[i * B_LOC:(i + 1) * B_LOC])}
            for i in range(N_CORES)]


def kernel(x: np.ndarray) -> np.ndarray:
    from concourse import bass_utils

    assert x.shape == (B, C, H, W), x.shape

    nc = _get_nc()
    res = bass_utils.run_bass_kernel_spmd(nc, _in_maps(x),
                                          core_ids=list(range(N_CORES)))
    out = np.concatenate(
        [res.results[i]["y"].astype(np.float32) for i in range(N_CORES)],
        axis=0)
    return out


# revision 12
# speedup vs baseline: 1.0154x; 1.0154x over previous
"""Trainium2 Bass kernel for InterpolativeUpsampler.

Op: nearest 2x upsample (H, W) followed by depthwise 3x3 blur
([1,2,1] outer [1,2,1] / 16, padding=1) on NCHW fp32.

The composite op is separable per axis:
    out[2i]   = (x[i-1] + 3*x[i]) / 4      (x[-1] = 0)
    out[2i+1] = (3*x[i] + x[i+1]) / 4      (x[H]  = 0)

Strategy: pure data parallel over batch (16 samples -> 8 cores, 2 each).
Per core: channels (128) on SBUF partitions; H tiled with 1-row halo.

Memory regime: input is prescaled by 1/16, zero-padded by 1 on H/W and
cast to fp16 on the host; the device writes fp16 output that the host
casts back to fp32. This halves HBM traffic vs fp32 (rel err ~1e-3,
gate is 2e-2) and makes every tile uniform (no edge cases on device).

Compute split (scalar_tensor_tensor has no fast DVE modes; tensor_tensor
gets 2x and tensor_scalar 4x for packed unit-stride fp16):
  ACT:    q3 = 3*x (feeds gpsimd) and t3 = 3*y (feeds the H adds)
  GPSIMD: W-pass even columns   y[2j]   = q3[j] + x[j-1]   (tensor_add)
  DVE:    W-pass odd columns    y[2j+1] = 3*x[j] + x[j+1]  (STT, 1x)
          H-pass                out[2i]   = t3[i] + y[i-1]  (TT, 2x)
                                out[2i+1] = t3[i] + y[i+1]  (TT, 2x)
  SP:     input DMA; ACT: output DMA.
"""

import numpy as np

B, C, H, W = 16, 128, 128, 128
N_CORES = 8
B_LOC = B // N_CORES      # samples per core
HB = 16                   # input rows per h-tile
NT = H // HB              # h-tiles per sample
R = HB + 2                # rows incl halo
WP = W + 2                # padded width

_cache = {}


def _build(opts: dict | None = None):
    import concourse.bacc as bacc
    import concourse.mybir as mybir
    import concourse.tile as tile

    o = {
        "q3_eng": "vector",       # q3 = 3*x (TS 4x mode; feeds both W adds)
        "w_even_eng": "gpsimd",   # W-pass even-col add (strided write)
        "w_odd_eng": "vector",    # W-pass odd-col add (strided write, 1x)
        "t3_eng": "scalar",       # t3 = 3*y
        "h_even_eng": "vector",   # H-pass adds (fp16 2x mode)
        "h_odd_eng": "vector",
        "in_dma_eng": "sync",
        "out_dma_eng": "scalar",
        "bufs_x": 3, "bufs_q": 3, "bufs_y": 4, "bufs_t": 2, "bufs_o": 3,
    }
    o.update(opts or {})

    f16 = mybir.dt.float16
    mult = mybir.AluOpType.mult
    add = mybir.AluOpType.add

    nc = bacc.Bacc("TRN2", target_bir_lowering=False, debug=False,
                   num_devices=N_CORES)
    eng = {"vector": nc.vector, "gpsimd": nc.gpsimd, "sync": nc.sync,
           "scalar": nc.scalar, "tensor": nc.tensor}
    x = nc.dram_tensor("x", [B_LOC, C, H + 2, WP], f16,
                       kind="ExternalInput").ap()
    y = nc.dram_tensor("y", [B_LOC, C, 2 * H, 2 * W], f16,
                       kind="ExternalOutput").ap()

    def emul(e, out, in_, s):
        """out = in_ * s on engine e (ACT activation or DVE/gpsimd TS)."""
        if e is nc.scalar:
            e.mul(out, in_, s)
        else:
            e.tensor_scalar_mul(out, in_, s)

    NTT = B_LOC * NT   # total tiles per core

    with tile.TileContext(nc) as tc:
        with tc.tile_pool(name="px", bufs=o["bufs_x"]) as px, \
             tc.tile_pool(name="pq", bufs=o["bufs_q"]) as pq, \
             tc.tile_pool(name="py", bufs=o["bufs_y"]) as py, \
             tc.tile_pool(name="pt", bufs=o["bufs_t"]) as pt, \
             tc.tile_pool(name="po", bufs=o["bufs_o"]) as po:
            live = {}

            def stage1(k):
                """Load tile k, W pass into yt (kept live for stage2)."""
                b, t = divmod(k, NT)
                h0 = t * HB
                xt = px.tile([C, R * WP], f16, name="xt")
                q3t = pq.tile([C, R * WP], f16, name="q3t")
                yt = py.tile([C, R * 2 * W], f16, name="yt")

                xv = xt.rearrange("c (r w) -> c r w", w=WP)
                qv = q3t.rearrange("c (r w) -> c r w", w=WP)
                yv2 = yt.rearrange("c (r w two) -> c r w two", w=W, two=2)

                # ---- load R=18 padded input rows (always uniform) ----
                eng[o["in_dma_eng"]].dma_start(xv, x[b][:, h0:h0 + R, :])
                # ---- W pass (x is pre-scaled by 1/16, zero-padded) ----
                # q3 = 3*x
                emul(eng[o["q3_eng"]], qv, xv, 3.0)
                # even: y[r, 2j] = 3*x[r, j] + x[r, j-1]
                eng[o["w_even_eng"]].tensor_add(
                    yv2[:, :, :, 0], qv[:, :, 1:W + 1], xv[:, :, 0:W])
                # odd: y[r, 2j+1] = 3*x[r, j] + x[r, j+1]
                eng[o["w_odd_eng"]].tensor_add(
                    yv2[:, :, :, 1], qv[:, :, 1:W + 1], xv[:, :, 2:W + 2])
                live[k] = yt

            def stage2(k):
                """t3, H pass, store for tile k (one iteration later)."""
                b, t = divmod(k, NT)
                h0 = t * HB
                yt = live.pop(k)
                t3 = pt.tile([C, HB * 2 * W], f16, name="t3")
                ot = po.tile([C, HB * 4 * W], f16, name="ot")

                yv = yt.rearrange("c (r w) -> c r w", w=2 * W)
                tv = t3.rearrange("c (r w) -> c r w", w=2 * W)
                ov = ot.rearrange("c (r two w) -> c r two w", two=2, w=2 * W)

                # ---- t3 = 3 * y (center rows) ----
                emul(eng[o["t3_eng"]], tv, yv[:, 1:HB + 1, :], 3.0)
                # ---- H pass: fp16 tensor_add (2x packed mode) ----
                # out[2i] = t3[i] + y[i-1]   (tile rows i -> yv row i+1)
                eng[o["h_even_eng"]].tensor_add(
                    ov[:, :, 0, :], tv[:, :, :], yv[:, 0:HB, :])
                # out[2i+1] = t3[i] + y[i+1]
                eng[o["h_odd_eng"]].tensor_add(
                    ov[:, :, 1, :], tv[:, :, :], yv[:, 2:HB + 2, :])
                # ---- store 2*HB output rows (contiguous in HBM) ----
                eng[o["out_dma_eng"]].dma_start(
                    y[b][:, 2 * h0:2 * h0 + 2 * HB, :],
                    ot.rearrange("c (h w) -> c h w", w=2 * W))

            # software pipeline: stage2 lags stage1 by one tile so each
            # engine's in-order queue interleaves W(k+1) with H(k)
            for k in range(NTT + 1):
                if k < NTT:
                    stage1(k)
                if k >= 1:
                    stage2(k - 1)

    nc.compile()
    return nc


def _get_nc():
    if "nc" not in _cache:
        _cache["nc"] = _build()
    return _cache["nc"]


def _in_maps(x: np.ndarray) -> list:
    xs = (np.asarray(x, dtype=np.float32) * (1.0 / 16.0)).astype(np.float16)
    xp = np.zeros((B, C, H + 2, WP), dtype=np.float16)
    xp[:, :, 1:H + 1, 1:W + 1] = xs
    return [{"x": np.ascontiguousarray(xp[i * B_LOC:(i + 1) * B_LOC])}
            for i in range(N_CORES)]


def kernel(x: np.ndarray) -> np.ndarray:
    from concourse import bass_utils

    assert x.shape == (B, C, H, W), x.shape

    nc = _get_nc()
    res = bass_utils.run_bass_kernel_spmd(nc, _in_maps(x),
                                          core_ids=list(range(N_CORES)))
    out = np.concatenate(
        [res.results[i]["y"].astype(np.float32) for i in range(N_CORES)],
        axis=0)
    return out


# revision 13
# speedup vs baseline: 1.0957x; 1.0790x over previous
"""Trainium2 Bass kernel for InterpolativeUpsampler.

Op: nearest 2x upsample (H, W) followed by depthwise 3x3 blur
([1,2,1] outer [1,2,1] / 16, padding=1) on NCHW fp32.

The composite op is separable per axis:
    out[2i]   = (x[i-1] + 3*x[i]) / 4      (x[-1] = 0)
    out[2i+1] = (3*x[i] + x[i+1]) / 4      (x[H]  = 0)

Strategy: pure data parallel over batch (16 samples -> 8 cores, 2 each).
Per core: channels (128) on SBUF partitions; H tiled with 1-row halo.

Memory regime: input is prescaled by 1/16, zero-padded by 1 on H/W and
cast to fp16 on the host; the device writes fp16 output that the host
casts back to fp32. This halves HBM traffic vs fp32 (rel err ~1e-3,
gate is 2e-2) and makes every tile uniform (no edge cases on device).

Compute split (scalar_tensor_tensor has no fast DVE modes; tensor_tensor
gets 2x and tensor_scalar 4x for packed unit-stride fp16):
  ACT:    q3 = 3*x (feeds gpsimd) and t3 = 3*y (feeds the H adds)
  GPSIMD: W-pass even columns   y[2j]   = q3[j] + x[j-1]   (tensor_add)
  DVE:    W-pass odd columns    y[2j+1] = 3*x[j] + x[j+1]  (STT, 1x)
          H-pass                out[2i]   = t3[i] + y[i-1]  (TT, 2x)
                                out[2i+1] = t3[i] + y[i+1]  (TT, 2x)
  SP:     input DMA; ACT: output DMA.
"""

import numpy as np

B, C, H, W = 16, 128, 128, 128
N_CORES = 8
B_LOC = B // N_CORES      # samples per core
HB = 16                   # input rows per h-tile
NT = H // HB              # h-tiles per sample
R = HB + 2                # rows incl halo
WP = W + 2                # padded width

_cache = {}


def _build(opts: dict | None = None):
    import concourse.bacc as bacc
    import concourse.mybir as mybir
    import concourse.tile as tile

    o = {
        "q3_eng": "scalar",       # q3 = 3*x on ACT (own SBUF port; gpsimd
                                  # contends with DVE and is net-negative)
        "w_even_eng": "vector",   # W-pass even-col add (strided write)
        "w_odd_eng": "vector",    # W-pass odd-col add (strided write, 1x)
        "t3_eng": "scalar",       # t3 = 3*y
        "h_even_eng": "vector",   # H-pass adds (fp16 2x mode)
        "h_odd_eng": "vector",
        "in_dma_eng": "sync",
        "out_dma_eng": "scalar",
        "bufs_x": 3, "bufs_q": 3, "bufs_y": 4, "bufs_t": 2, "bufs_o": 3,
    }
    o.update(opts or {})

    f16 = mybir.dt.float16
    mult = mybir.AluOpType.mult
    add = mybir.AluOpType.add

    nc = bacc.Bacc("TRN2", target_bir_lowering=False, debug=False,
                   num_devices=N_CORES)
    eng = {"vector": nc.vector, "gpsimd": nc.gpsimd, "sync": nc.sync,
           "scalar": nc.scalar, "tensor": nc.tensor}
    x = nc.dram_tensor("x", [B_LOC, C, H + 2, WP], f16,
                       kind="ExternalInput").ap()
    y = nc.dram_tensor("y", [B_LOC, C, 2 * H, 2 * W], f16,
                       kind="ExternalOutput").ap()

    def emul(e, out, in_, s):
        """out = in_ * s on engine e (ACT activation or DVE/gpsimd TS)."""
        if e is nc.scalar:
            e.mul(out, in_, s)
        else:
            e.tensor_scalar_mul(out, in_, s)

    NTT = B_LOC * NT   # total tiles per core

    with tile.TileContext(nc) as tc:
        with tc.tile_pool(name="px", bufs=o["bufs_x"]) as px, \
             tc.tile_pool(name="pq", bufs=o["bufs_q"]) as pq, \
             tc.tile_pool(name="py", bufs=o["bufs_y"]) as py, \
             tc.tile_pool(name="pt", bufs=o["bufs_t"]) as pt, \
             tc.tile_pool(name="po", bufs=o["bufs_o"]) as po:
            live = {}

            def stage1(k):
                """Load tile k, W pass into yt (kept live for stage2)."""
                b, t = divmod(k, NT)
                h0 = t * HB
                xt = px.tile([C, R * WP], f16, name="xt")
                q3t = pq.tile([C, R * WP], f16, name="q3t")
                yt = py.tile([C, R * 2 * W], f16, name="yt")

                xv = xt.rearrange("c (r w) -> c r w", w=WP)
                qv = q3t.rearrange("c (r w) -> c r w", w=WP)
                yv2 = yt.rearrange("c (r w two) -> c r w two", w=W, two=2)

                # ---- load R=18 padded input rows (always uniform) ----
                eng[o["in_dma_eng"]].dma_start(xv, x[b][:, h0:h0 + R, :])
                # ---- W pass (x is pre-scaled by 1/16, zero-padded) ----
                # q3 = 3*x
                emul(eng[o["q3_eng"]], qv, xv, 3.0)
                # even: y[r, 2j] = 3*x[r, j] + x[r, j-1]
                eng[o["w_even_eng"]].tensor_add(
                    yv2[:, :, :, 0], qv[:, :, 1:W + 1], xv[:, :, 0:W])
                # odd: y[r, 2j+1] = 3*x[r, j] + x[r, j+1]
                eng[o["w_odd_eng"]].tensor_add(
                    yv2[:, :, :, 1], qv[:, :, 1:W + 1], xv[:, :, 2:W + 2])
                live[k] = yt

            def stage2(k):
                """t3, H pass, store for tile k (one iteration later)."""
                b, t = divmod(k, NT)
                h0 = t * HB
                yt = live.pop(k)
                t3 = pt.tile([C, HB * 2 * W], f16, name="t3")
                ot = po.tile([C, HB * 4 * W], f16, name="ot")

                yv = yt.rearrange("c (r w) -> c r w", w=2 * W)
                tv = t3.rearrange("c (r w) -> c r w", w=2 * W)
                ov = ot.rearrange("c (r two w) -> c r two w", two=2, w=2 * W)

                # ---- t3 = 3 * y (center rows) ----
                emul(eng[o["t3_eng"]], tv, yv[:, 1:HB + 1, :], 3.0)
                # ---- H pass: fp16 tensor_add (2x packed mode) ----
                # out[2i] = t3[i] + y[i-1]   (tile rows i -> yv row i+1)
                eng[o["h_even_eng"]].tensor_add(
                    ov[:, :, 0, :], tv[:, :, :], yv[:, 0:HB, :])
                # out[2i+1] = t3[i] + y[i+1]
                eng[o["h_odd_eng"]].tensor_add(
                    ov[:, :, 1, :], tv[:, :, :], yv[:, 2:HB + 2, :])
                # ---- store 2*HB output rows (contiguous in HBM) ----
                eng[o["out_dma_eng"]].dma_start(
                    y[b][:, 2 * h0:2 * h0 + 2 * HB, :],
                    ot.rearrange("c (h w) -> c h w", w=2 * W))

            # software pipeline: stage2 lags stage1 by one tile so each
            # engine's in-order queue interleaves W(k+1) with H(k)
            for k in range(NTT + 1):
                if k < NTT:
                    stage1(k)
                if k >= 1:
                    stage2(k - 1)

    nc.compile()
    return nc


def _get_nc():
    if "nc" not in _cache:
        _cache["nc"] = _build()
    return _cache["nc"]


def _in_maps(x: np.ndarray) -> list:
    xs = (np.asarray(x, dtype=np.float32) * (1.0 / 16.0)).astype(np.float16)
    xp = np.zeros((B, C, H + 2, WP), dtype=np.float16)
    xp[:, :, 1:H + 1, 1:W + 1] = xs
    return [{"x": np.ascontiguousarray(xp[i * B_LOC:(i + 1) * B_LOC])}
            for i in range(N_CORES)]


def kernel(x: np.ndarray) -> np.ndarray:
    from concourse import bass_utils

    assert x.shape == (B, C, H, W), x.shape

    nc = _get_nc()
    res = bass_utils.run_bass_kernel_spmd(nc, _in_maps(x),
                                          core_ids=list(range(N_CORES)))
    out = np.concatenate(
        [res.results[i]["y"].astype(np.float32) for i in range(N_CORES)],
        axis=0)
    return out


# revision 16
# speedup vs baseline: 1.3112x; 1.1967x over previous
"""Trainium2 Bass kernel for InterpolativeUpsampler.

Op: nearest 2x upsample (H, W) followed by depthwise 3x3 blur
([1,2,1] outer [1,2,1] / 16, padding=1) on NCHW fp32.

The composite op is separable per axis:
    out[2i]   = (x[i-1] + 3*x[i]) / 4      (x[-1] = 0)
    out[2i+1] = (3*x[i] + x[i+1]) / 4      (x[H]  = 0)

Strategy: pure data parallel over batch (16 samples -> 8 cores, 2 each).
Per core: channels (128) on SBUF partitions; H tiled with 1-row halo.

Memory regime: input is prescaled by 1/16, zero-padded by 1 on H/W and
cast to fp16 on the host; the device writes fp16 output that the host
casts back to fp32. This halves HBM traffic vs fp32 (rel err ~1e-3,
gate is 2e-2) and makes every tile uniform (no edge cases on device).

Compute split (scalar_tensor_tensor has no fast DVE modes; tensor_tensor
gets 2x and tensor_scalar 4x for packed unit-stride fp16):
  ACT:    q3 = 3*x (feeds gpsimd) and t3 = 3*y (feeds the H adds)
  GPSIMD: W-pass even columns   y[2j]   = q3[j] + x[j-1]   (tensor_add)
  DVE:    W-pass odd columns    y[2j+1] = 3*x[j] + x[j+1]  (STT, 1x)
          H-pass                out[2i]   = t3[i] + y[i-1]  (TT, 2x)
                                out[2i+1] = t3[i] + y[i+1]  (TT, 2x)
  SP:     input DMA; ACT: output DMA.
"""

import numpy as np

B, C, H, W = 16, 128, 128, 128
N_CORES = 8
B_LOC = B // N_CORES      # samples per core
HB = 16                   # input rows per h-tile
NT = H // HB              # h-tiles per sample
R = HB + 2                # rows incl halo
WP = W + 2                # padded width

_cache = {}


def _build(opts: dict | None = None):
    import concourse.bacc as bacc
    import concourse.mybir as mybir
    import concourse.tile as tile

    o = {
        "w_eng": "vector",        # merged W-pass STT (strided writes, 1x)
        "t3_eng": "scalar",       # t3 = 3*y
        "h_eng": "vector",        # merged H-pass add (fp16 2x mode)
        "in_dma_eng": "sync",
        "out_dma_eng": "scalar",
        "bufs_x": 3, "bufs_y": 4, "bufs_t": 2, "bufs_o": 3,
    }
    o.update(opts or {})

    f16 = mybir.dt.float16
    mult = mybir.AluOpType.mult
    add = mybir.AluOpType.add

    nc = bacc.Bacc("TRN2", target_bir_lowering=False, debug=False,
                   num_devices=N_CORES)
    eng = {"vector": nc.vector, "gpsimd": nc.gpsimd, "sync": nc.sync,
           "scalar": nc.scalar, "tensor": nc.tensor}
    x = nc.dram_tensor("x", [B_LOC, C, H + 2, WP], f16,
                       kind="ExternalInput").ap()
    y = nc.dram_tensor("y", [B_LOC, C, 2 * H, 2 * W], f16,
                       kind="ExternalOutput").ap()

    def emul(e, out, in_, s):
        """out = in_ * s on engine e (ACT activation or DVE/gpsimd TS)."""
        if e is nc.scalar:
            e.mul(out, in_, s)
        else:
            e.tensor_scalar_mul(out, in_, s)

    NTT = B_LOC * NT   # total tiles per core

    from concourse.bass import AP

    with tile.TileContext(nc) as tc:
        with tc.tile_pool(name="px", bufs=o["bufs_x"]) as px, \
             tc.tile_pool(name="py", bufs=o["bufs_y"]) as py, \
             tc.tile_pool(name="pt", bufs=o["bufs_t"]) as pt, \
             tc.tile_pool(name="po", bufs=o["bufs_o"]) as po:
            live = {}

            def stage1(k):
                """Load tile k, W pass into yt (kept live for stage2)."""
                b, t = divmod(k, NT)
                h0 = t * HB
                xt = px.tile([C, R * WP], f16, name="xt")
                yt = py.tile([C, R * 2 * W], f16, name="yt")

                xv = xt.rearrange("c (r w) -> c r w", w=WP)
                yv2 = yt.rearrange("c (r w two) -> c r w two", w=W, two=2)

                # ---- load R=18 padded input rows (always uniform) ----
                eng[o["in_dma_eng"]].dma_start(xv, x[b][:, h0:h0 + R, :])
                # ---- W pass (x is pre-scaled by 1/16, zero-padded) ----
                # y[r, 2j+p] = 3*x[r, j] + x[r, j-1+2p]; one STT per
                # parity (walrus limits STT operands to 3D access patterns)
                eng[o["w_eng"]].scalar_tensor_tensor(
                    yv2[:, :, :, 0], xv[:, :, 1:W + 1], 3.0,
                    xv[:, :, 0:W], op0=mult, op1=add)
                eng[o["w_eng"]].scalar_tensor_tensor(
                    yv2[:, :, :, 1], xv[:, :, 1:W + 1], 3.0,
                    xv[:, :, 2:W + 2], op0=mult, op1=add)
                live[k] = yt

            def stage2(k):
                """t3, H pass, store for tile k (one iteration later)."""
                b, t = divmod(k, NT)
                h0 = t * HB
                yt = live.pop(k)
                t3 = pt.tile([C, HB * 2 * W], f16, name="t3")
                ot = po.tile([C, HB * 4 * W], f16, name="ot")

                yv = yt.rearrange("c (r w) -> c r w", w=2 * W)
                tv = t3.rearrange("c (r w) -> c r w", w=2 * W)
                ov = ot.rearrange("c (r two w) -> c r two w", two=2, w=2 * W)

                # ---- t3 = 3 * y (center rows) ----
                emul(eng[o["t3_eng"]], tv, yv[:, 1:HB + 1, :], 3.0)
                # ---- H pass: out[2i+p] = t3[i] + y[i-1+2p], ONE fp16
                # tensor_add in 2x packed mode (all last dims unit-stride):
                # in0 broadcasts t3 over the row-parity dim, in1 steps 2
                # rows over it (y rows i-1 and i+1 at tile rows i, i+2).
                tvb = tv.unsqueeze(2).broadcast_to((C, HB, 2, 2 * W))
                ynb = AP(yv.tensor, yv.offset,
                         [list(yv.ap[0]), [2 * W, HB], [4 * W, 2],
                          [1, 2 * W]])
                eng[o["h_eng"]].tensor_add(ov[:, :, :, :], tvb, ynb)
                # ---- store 2*HB output rows (contiguous in HBM) ----
                eng[o["out_dma_eng"]].dma_start(
                    y[b][:, 2 * h0:2 * h0 + 2 * HB, :],
                    ot.rearrange("c (h w) -> c h w", w=2 * W))

            # software pipeline: stage2 lags stage1 by one tile so each
            # engine's in-order queue interleaves W(k+1) with H(k)
            for k in range(NTT + 1):
                if k < NTT:
                    stage1(k)
                if k >= 1:
                    stage2(k - 1)

    nc.compile()
    return nc


def _get_nc():
    if "nc" not in _cache:
        _cache["nc"] = _build()
    return _cache["nc"]


def _in_maps(x: np.ndarray) -> list:
    xs = (np.asarray(x, dtype=np.float32) * (1.0 / 16.0)).astype(np.float16)
    xp = np.zeros((B, C, H + 2, WP), dtype=np.float16)
    xp[:, :, 1:H + 1, 1:W + 1] = xs
    return [{"x": np.ascontiguousarray(xp[i * B_LOC:(i + 1) * B_LOC])}
            for i in range(N_CORES)]


def kernel(x: np.ndarray) -> np.ndarray:
    from concourse import bass_utils

    assert x.shape == (B, C, H, W), x.shape

    nc = _get_nc()
    res = bass_utils.run_bass_kernel_spmd(nc, _in_maps(x),
                                          core_ids=list(range(N_CORES)))
    out = np.concatenate(
        [res.results[i]["y"].astype(np.float32) for i in range(N_CORES)],
        axis=0)
    return out
